# revision 22
# baseline (speedup 1.0000x reference)
"""Trainium2 Bass kernel for BottleneckAttention (patch attention).

q patches [160, 5120] from z1_hat (non-overlapping 10x4 unfold),
kv patches [5551, 5120] from z2 (overlapping unfold, Hk=91 x Wk=61),
scores = q @ kv.T / 5120, softmax over kv patches, out = attn @ kv,
folded back to [1, 128, 100, 64].

Sharding: 12 kv h-rows (768 flat positions) per core; every core computes
all 160 q columns; host combines with an all-gather softmax using the
centered form f = e - 1 (the exact colsum term is added in fp64 on host).

Per-core kernel (v3): every matmul uses the full 128-wide PE array; all
matmul operands are fp8e4 (~1.6e-3 relative error vs the 2e-2 budget).
  phase 1 computes scores TRANSPOSED [pos, q]: block pairs are swept
    ij-outer (two interleaved accumulation chains) so the q DMA quarters
    stream in behind the compute. Stationary operand = z2-slab windows
    (implicit convolution; no kv matrix is ever materialized or DMAed).
  exp on ScalarE (scale 1/5120), then f = (e-1)*mask on VectorE (mask
    zeroes the invalid w>=61 / h>=91 positions on-device; fp8 f).
  phase 2 computes out TRANSPOSED [(c,i,j), q]: per-tap 6-chunk chains
    with partition-phase-shifted copies of z2T as stationary operand.
    PSUM->SBUF copies alternate ScalarE/VectorE; fp16 output.
  denominator = ones-vector matmul, slotted after tile 31 so its DMA
    overlaps the remaining tiles.
All input DMAs ride one in-order queue (Activation engine HW-DGE) with a
single arrival semaphore; outputs go on the SP queue. Semaphore count is
minimized because the end-of-kernel reset postamble is ~127ns/semaphore
on the PE.
"""

import sys

sys.path.insert(0, "/opt/trn_rl_repo")

import numpy as np
import ml_dtypes

import concourse.bass as bass
import concourse.mybir as mybir

DT = mybir.dt
AF = mybir.ActivationFunctionType
ALU = mybir.AluOpType

# problem geometry (hardcoded from the reference module)
KC, KH, KW = 128, 10, 4
H, W = 100, 64
NH, NW = H // KH, W // KW          # 10, 16
PQ = NH * NW                       # 160 q patches
D = KC * KH * KW                   # 5120
HK, WK = H - KH + 1, W - KW + 1    # 91, 61
NCORES = 8
HPC = 12                           # kv h-rows per core
NPOS = 24 * W                      # 1536 slab positions per core
NOWN = HPC * W                     # 768 owned positions per core
NB = NOWN // 128                   # 6 position blocks
NIJ = KH * KW                      # 40 (i,j) taps
QPAD = 256                         # q/f sbuf free-dim pad for alignment
PHASES = (0, 1, 2, 3, 64, 65, 66, 67)
NK = 10                            # ztp chunks per phase
SCALE = 1.0 / D
ZSPLIT = 896                       # zc first-chunk boundary

F8 = ml_dtypes.float8_e4m3

_CACHE = {}

# output DMA chunk boundaries (tiles) — small tail chunk
OCH = [(0, 8), (8, 16), (16, 24), (24, 32), (32, 38), (38, 40)]


def _build_nc():
    nc = bass.Bass()
    zc_d = nc.declare_dram_parameter("zc", [KC, NPOS], DT.float8e4, isOutput=False)
    q_d = nc.declare_dram_parameter("qT3", [KC, NIJ, PQ], DT.float8e4, isOutput=False)
    zt_d = nc.declare_dram_parameter(
        "ztp", [128, len(PHASES), NK, KC], DT.float8e4, isOutput=False
    )
    mk_d = nc.declare_dram_parameter("msk", [128, 8], DT.float32, isOutput=False)
    out_d = nc.declare_dram_parameter("out", [KC, NIJ, PQ], DT.float16, isOutput=True)
    den_d = nc.declare_dram_parameter("den", [1, PQ], DT.float32, isOutput=True)

    from contextlib import ExitStack

    ctx = ExitStack()
    with ctx:
        zc_sb = ctx.enter_context(nc.sbuf_tensor([KC, NPOS], DT.float8e4))
        q_sb = ctx.enter_context(nc.sbuf_tensor([KC, NIJ, PQ], DT.float8e4))
        zt_sb = ctx.enter_context(
            nc.sbuf_tensor([128, len(PHASES), NK, KC], DT.float8e4)
        )
        mk_sb = ctx.enter_context(nc.sbuf_tensor([128, 8], DT.float32))
        e_sb = ctx.enter_context(nc.sbuf_tensor([128, NB, 192], DT.float32))
        f_sb = ctx.enter_context(nc.sbuf_tensor([128, NB, QPAD], DT.float8e4))
        o_sb = ctx.enter_context(nc.sbuf_tensor([128, NIJ, PQ], DT.float16))
        den_sb = ctx.enter_context(nc.sbuf_tensor([1, 192], DT.float32))
        ones_sb = ctx.enter_context(nc.sbuf_tensor([128, 1], DT.float8e4))
        wz = ctx.enter_context(nc.sbuf_tensor([128, 128], DT.float8e4))

        ps = [
            ctx.enter_context(nc.psum_tensor(f"ps{i}", [128, 512], DT.float32))
            for i in range(8)
        ]

        s_wz = ctx.enter_context(nc.semaphore("s_wz"))
        s_i0 = ctx.enter_context(nc.semaphore("s_i0"))
        s_q00 = ctx.enter_context(nc.semaphore("s_q00"))
        s_i01 = ctx.enter_context(nc.semaphore("s_i01"))
        s_i1 = ctx.enter_context(nc.semaphore("s_i1"))
        s_i2 = ctx.enter_context(nc.semaphore("s_i2"))
        s_i3 = ctx.enter_context(nc.semaphore("s_i3"))
        s_im = ctx.enter_context(nc.semaphore("s_im"))
        s_iz = ctx.enter_context(nc.semaphore("s_iz"))
        s_p = ctx.enter_context(nc.semaphore("s_p"))
        s_cpa = ctx.enter_context(nc.semaphore("s_cpa"))
        s_cpv = ctx.enter_context(nc.semaphore("s_cpv"))
        s_f = ctx.enter_context(nc.semaphore("s_f"))
        s_o = ctx.enter_context(nc.semaphore("s_o"))

        # s_p schedule: pairs 1..6, tiles 0..31 -> 7..38, den -> 39,
        #               tiles 32..39 -> 40..47
        # s_cpa schedule: exps 1..6, even-tile copies 7..26, den copy 27
        # s_cpv schedule: odd-tile copy of tile 2m+1 -> m+1 (1..20)
        def sp_tile(g):
            return 7 + g if g < 35 else 8 + g

        with nc.Block() as block:

            @block.sync
            def _(sync):
                # outputs only; inputs ride the Activation queue
                for a, b in OCH[:4]:
                    ev = 6 + (b + 1) // 2
                    sync.wait_ge(s_cpa, ev)
                    sync.wait_ge(s_cpv, b // 2)
                    sl = slice(a, b)
                    sync.dma_start(out_d[:, sl, :], o_sb[:, sl, :]).then_inc(
                        s_o, 16
                    )
                sync.wait_ge(s_cpa, 25)
                sync.dma_start(den_d[:, :], den_sb[0:1, 0:PQ]).then_inc(s_o, 16)
                sync.wait_ge(s_o, 112)

            @block.tensor
            def _(pe):
                # HAM warmup on the zeroed wz tile while input DMAs land
                pe.wait_ge(s_wz, 1)
                for w_ in range(20):
                    nc.tensor.matmul(
                        ps[7][0:128, 0:128],
                        wz[:, 0:128],
                        wz[:, 0:128],
                        start=(w_ == 0),
                        stop=(w_ == 19),
                    )
                pe.wait_ge(s_i0, 16)  # zc_a landed; keep PE hot on it
                for w_ in range(10):
                    nc.tensor.matmul(
                        ps[7][0:128, 0:512],
                        zc_sb[:, 0:128],
                        zc_sb[:, 0:512],
                        start=(w_ == 0),
                        stop=(w_ == 9),
                    )
                pe.wait_ge(s_q00, 16)  # + q00
                # phase 1: block pairs, ij-outer (two interleaved chains)
                for pr in range(NB // 2):
                    b0, b1 = 2 * pr, 2 * pr + 1
                    for ij in range(NIJ):
                        if pr == 0 and ij == 4:
                            pe.wait_ge(s_i01, 16)  # q taps 4..9
                        elif pr == 0 and ij == 10:
                            pe.wait_ge(s_i1, 32)  # zc_b + q1
                        elif pr == 0 and ij == 20:
                            pe.wait_ge(s_i2, 16)
                        elif pr == 0 and ij == 30:
                            pe.wait_ge(s_i3, 16)
                        i_, j_ = ij // KW, ij % KW
                        d0 = 64 * i_ + j_
                        mmA = nc.tensor.matmul(
                            ps[b0][0:128, 0:PQ],
                            zc_sb[:, 128 * b0 + d0 : 128 * b0 + d0 + 128],
                            q_sb[:, ij, 0:PQ],
                            start=(ij == 0),
                            stop=(ij == NIJ - 1),
                        )
                        mmB = nc.tensor.matmul(
                            ps[b1][0:128, 0:PQ],
                            zc_sb[:, 128 * b1 + d0 : 128 * b1 + d0 + 128],
                            q_sb[:, ij, 0:PQ],
                            start=(ij == 0),
                            stop=(ij == NIJ - 1),
                        )
                    mmA.then_inc(s_p, 1)
                    mmB.then_inc(s_p, 1)
                # phase 2: out_T[(c,i,j), q] per tap, 6-chunk chains
                pe.wait_ge(s_iz, 32)  # ztp resident
                for g in range(NIJ):
                    i_, j_ = g // KW, g % KW
                    fi = (i_ % 2) * 4 + j_
                    k0 = i_ // 2
                    if g == 1:
                        pe.wait_ge(s_f, NB)
                    elif g >= 8 and g % 4 == 0:
                        pe.wait_ge(s_cpa, g // 2 + 4)
                        pe.wait_ge(s_cpv, g // 2 - 2)
                    if g == 39:
                        pe.wait_ge(s_cpa, 25)  # den copied out of ps[7]
                    for b in range(NB):
                        if g == 0:
                            pe.wait_ge(s_f, b + 1)
                        mm = nc.tensor.matmul(
                            ps[g % 8][0:128, 0:PQ],
                            zt_sb[:, fi, k0 + b, :],
                            f_sb[:, b, 0:PQ],
                            start=(b == 0),
                            stop=(b == NB - 1),
                        )
                    mm.then_inc(s_p, 1)
                    if g == 34:
                        # denominator: ones.T @ f -> [1, 160] in ps[7]
                        pe.wait_ge(s_wz, 2)   # ones ready
                        pe.wait_ge(s_cpv, 16)  # ps[7] freed (tile 31 copy)
                        for b in range(NB):
                            mm = nc.tensor.matmul(
                                ps[7][0:1, 0:PQ],
                                ones_sb[0:128, 0:1],
                                f_sb[:, b, 0:PQ],
                                start=(b == 0),
                                stop=(b == NB - 1),
                            )
                        mm.then_inc(s_p, 1)  # s_p = 39

            @block.scalar
            def _(act):
                # input DMAs on the Activation HW-DGE queue, consumption order
                act.dma_start(zc_sb[:, 0:ZSPLIT], zc_d[:, 0:ZSPLIT]).then_inc(s_i0, 16)
                act.dma_start(q_sb[:, 0:4, 0:PQ], q_d[:, 0:4, :]).then_inc(s_q00, 16)
                act.dma_start(q_sb[:, 4:10, 0:PQ], q_d[:, 4:10, :]).then_inc(s_i01, 16)
                act.dma_start(zc_sb[:, ZSPLIT:], zc_d[:, ZSPLIT:]).then_inc(s_i1, 16)
                act.dma_start(q_sb[:, 10:20, 0:PQ], q_d[:, 10:20, :]).then_inc(s_i1, 16)
                act.dma_start(q_sb[:, 20:30, 0:PQ], q_d[:, 20:30, :]).then_inc(s_i2, 16)
                act.dma_start(q_sb[:, 30:40, 0:PQ], q_d[:, 30:40, :]).then_inc(s_i3, 16)
                act.dma_start(mk_sb[:], mk_d[:]).then_inc(s_im, 16)
                for h in range(2):
                    sl = slice(4 * h, 4 * h + 4)
                    act.dma_start(zt_sb[:, sl], zt_d[:, sl]).then_inc(s_iz, 16)
                for b in range(NB):
                    act.wait_ge(s_p, b + 1)
                    nc.scalar.activation(
                        e_sb[:, b, 0:PQ], ps[b][0:128, 0:PQ], AF.Exp, scale=SCALE
                    ).then_inc(s_cpa, 1)  # 1..6
                for g in range(0, NIJ, 2):
                    act.wait_ge(s_p, sp_tile(g))
                    nc.scalar.activation(
                        o_sb[:, g, :], ps[g % 8][0:128, 0:PQ], AF.Copy
                    ).then_inc(s_cpa, 1)  # evens<=34: 7..24, 36+: 26..27
                    if g == 34:
                        act.wait_ge(s_p, 42)
                        nc.scalar.activation(
                            den_sb[0:1, 0:PQ], ps[7][0:1, 0:PQ], AF.Copy
                        ).then_inc(s_cpa, 1)  # 25
                    if g == 36:
                        act.wait_ge(s_cpa, 26)  # own copies (sim dep credit)
                        act.wait_ge(s_cpv, 19)  # odd tiles through 37
                        act.dma_start(
                            out_d[:, 32:38, :], o_sb[:, 32:38, :]
                        ).then_inc(s_o, 16)
                    if g == 38:
                        act.wait_ge(s_cpa, 27)  # own copies (sim dep credit)
                        act.wait_ge(s_cpv, 20)  # tile 39 copied
                        act.dma_start(
                            out_d[:, 38:40, :], o_sb[:, 38:40, :]
                        ).then_inc(s_o, 16)

            @block.vector
            def _(dve):
                nc.vector.memset(wz[:], 0.0).then_inc(s_wz, 1)
                nc.vector.memset(ones_sb[:], 1.0).then_inc(s_wz, 1)
                dve.wait_ge(s_im, 16)  # mask resident
                for b in range(NB):
                    dve.wait_ge(s_cpa, b + 1)
                    nc.vector.tensor_scalar(
                        f_sb[:, b, 0:PQ],
                        e_sb[:, b, 0:PQ],
                        -1.0,
                        mk_sb[:, b : b + 1],
                        ALU.add,
                        ALU.mult,
                    ).then_inc(s_f, 1)
                for g in range(1, NIJ, 2):
                    dve.wait_ge(s_p, sp_tile(g))
                    nc.vector.tensor_copy(
                        o_sb[:, g, :], ps[g % 8][0:128, 0:PQ]
                    ).then_inc(s_cpv, 1)

    return nc


def _host_prep(z1_hat, z2):
    z1 = np.asarray(z1_hat, dtype=np.float32)[0]   # [128, 100, 64]
    z2a = np.asarray(z2, dtype=np.float32)[0]

    q = z1.reshape(KC, NH, KH, NW, KW).transpose(1, 3, 0, 2, 4).reshape(PQ, D)
    qT3 = np.ascontiguousarray(
        q.reshape(PQ, KC, NIJ).transpose(1, 2, 0).astype(F8)
    )

    z_pad = np.zeros((KC, 112, W), dtype=np.float32)
    z_pad[:, :H] = z2a

    in_maps = []
    for core in range(NCORES):
        h0 = HPC * core
        slab = z_pad[:, h0 : h0 + 24, :]                  # [128, 24, 64]
        zc = np.ascontiguousarray(slab.reshape(KC, NPOS).astype(F8))
        z2T_pad = np.zeros((NPOS + 128, KC), dtype=np.float32)
        z2T_pad[:NPOS] = slab.reshape(KC, NPOS).T
        ztp = np.zeros((128, len(PHASES), NK, KC), dtype=F8)
        for fi, ph in enumerate(PHASES):
            v = z2T_pad[ph : ph + NK * 128].reshape(NK, 128, KC)
            ztp[:, fi] = v.transpose(1, 0, 2).astype(F8)
        x = np.arange(NOWN)
        real = ((x % W) < WK) & ((h0 + x // W) < HK)
        msk = np.zeros((128, 8), dtype=np.float32)
        msk[:, :NB] = real.reshape(NB, 128).T
        in_maps.append(
            {
                "zc": zc,
                "qT3": qT3,
                "ztp": np.ascontiguousarray(ztp),
                "msk": msk,
            }
        )

    # colsum[(c,i,j)] = sum of kv rows over real patches, via integral image
    I = np.zeros((KC, H + 1, W + 1), dtype=np.float64)
    I[:, 1:, 1:] = z2a.astype(np.float64).cumsum(axis=1).cumsum(axis=2)
    colsum = np.zeros((KC, KH, KW), dtype=np.float64)
    for i in range(KH):
        for j in range(KW):
            colsum[:, i, j] = (
                I[:, i + HK, j + WK] - I[:, i, j + WK] - I[:, i + HK, j] + I[:, i, j]
            )
    return in_maps, colsum.reshape(KC, NIJ)


def kernel(z1_hat, z2):
    from concourse.bass_utils import run_bass_kernel_spmd

    in_maps, colsum = _host_prep(z1_hat, z2)
    if "nc" not in _CACHE:
        _CACHE["nc"] = _build_nc()
    nc = _CACHE["nc"]
    res = run_bass_kernel_spmd(nc, in_maps, list(range(NCORES)))
    num = colsum[:, :, None].astype(np.float64).copy()     # [128, 40, 1]
    num = np.broadcast_to(num, (KC, NIJ, PQ)).copy()
    den = np.full((PQ,), float(HK * WK), dtype=np.float64)
    for r in res.results:
        num += r["out"].astype(np.float64)
        den += r["den"].astype(np.float64)[0]
    out = num / den[None, None, :]
    # fold: [c, (i,j), q=(nh,nw)] -> [1, 128, 100, 64]
    arr = out.reshape(KC, KH, KW, NH, NW).transpose(0, 3, 1, 4, 2)
    return np.ascontiguousarray(arr.reshape(1, KC, H, W).astype(np.float32))
